# revision 11
# baseline (speedup 1.0000x reference)
"""Trainium2 Bass kernel for nn_Decoder (gnn_message_passing).

Mathematical simplification of the reference (verified exact vs the jax oracle):
the reference's inner scan collapses — only the immediate predecessor (idx-1)
contributes to message aggregation, hv_new is invariant across inner steps, and
edge decisions reduce to per-node dot products d1[j]=hv_j.w1, d2[j]=hv_j.w2
thresholded at sigmoid>=0.5.

Per outer step idx (batch-on-partitions layout, 64 batch rows/core):
  enc[idx] = softmax(gs @ Wvert.T + bvert)          (gs = hv_{idx-1}, gs0 = z@Wlin1.T+blin1)
  a        = dep[:,idx,idx-1] * hv_{idx-1}           (idx>=1)
  h_in     = 15*sigmoid(b_gate)*b_map + sigmoid(a@Wg.T+bg) * (a@Wm.T+bm)   (h_in=gs0 at idx=0)
  gru gates from h_in and x_idx -> hv_idx
  d1[idx] = hv.w1, d2[idx] = hv.w2
edges[i,j] = step(d1[i]+d2[j]+be) for j<=i-2; edges[i,i-1] = step(d1[i-1]+d2[i-1]+be).

All matmuls run in fp32 (fp32r is an 11-bit-mantissa format — too coarse for the
hard edge thresholds). Biases enter via a constant ones-row appended to the
stationary operand (row 117 of the K-tile-3 slice) and bias rows baked into the
weight layouts, so no separate bias matmuls are needed.

Sharding: pure data parallel, batch 512 -> 64 per core across 8 cores.
"""

import numpy as np

B, S, H, C = 512, 16, 501, 7
NCORES = 8
BC = B // NCORES  # 64 batch rows per core
KT = [128, 128, 128, 117]    # K tiles over H=501 (data rows)
KTL = [128, 128, 128, 118]   # lhsT/rhs row counts (tile 3 includes the ones/bias row)
KOFF = [0, 128, 256, 384]
H3 = 3 * H  # 1503

# packed bias/constant row (1, 2H+1): b_gate | b_map | b_edge
BO_GATE, BO_MAP, BO_BE = 0, H, 2 * H
BIAS_LEN = 2 * H + 1

_CACHE = {}


def _build_module():
    import concourse.bass as bass
    import concourse.bacc as bacc
    import concourse.mybir as mybir
    from concourse.tile import TileContext

    f32 = mybir.dt.float32
    Alu = mybir.AluOpType
    Act = mybir.ActivationFunctionType
    Axis = mybir.AxisListType

    nc = bacc.Bacc("TRN2", target_bir_lowering=False, debug=False,
                   enable_asserts=False, num_devices=NCORES)

    # ---- DRAM I/O ----
    d_zT4 = nc.dram_tensor("zT4", [128, 256], f32, kind="ExternalInput")
    d_wlin = nc.dram_tensor("WlinT", [128, 4 * H], f32, kind="ExternalInput")
    d_wg = nc.dram_tensor("WgT", [128, 4 * H], f32, kind="ExternalInput")
    d_wm = nc.dram_tensor("WmT", [128, 4 * H], f32, kind="ExternalInput")
    d_whh = nc.dram_tensor("WhhT", [128, 4 * H3], f32, kind="ExternalInput")
    d_wih = nc.dram_tensor("WihT", [8, H3], f32, kind="ExternalInput")
    d_wv = nc.dram_tensor("WvT", [128, 4 * C], f32, kind="ExternalInput")
    d_w12 = nc.dram_tensor("w12", [128, 8], f32, kind="ExternalInput")
    d_xT = nc.dram_tensor("xT", [8, S * BC], f32, kind="ExternalInput")
    d_s4 = nc.dram_tensor("S4r", [128, 15 * 256], f32, kind="ExternalInput")
    d_eye = nc.dram_tensor("eye64", [64, 64], f32, kind="ExternalInput")
    d_mask = nc.dram_tensor("maskOD", [64, 256], f32, kind="ExternalInput")
    d_bias = nc.dram_tensor("BIASROW", [1, BIAS_LEN], f32, kind="ExternalInput")
    d_ones = nc.dram_tensor("onesrow", [1, 64], f32, kind="ExternalInput")
    d_odep = nc.dram_tensor("out_dep", [BC, 256], f32, kind="ExternalOutput")
    d_oenc = nc.dram_tensor("out_enc", [BC, S * C], f32, kind="ExternalOutput")

    def bcast(dram_handle, col0, ncols, nparts):
        ap = dram_handle.ap()
        return bass.AP(tensor=ap.tensor, offset=ap.offset + col0,
                       ap=[[0, nparts], [1, ncols]])

    with TileContext(nc) as tc:
        with (
            tc.tile_pool(name="const", bufs=1) as cp,
            tc.tile_pool(name="work", bufs=2) as wp,
            tc.tile_pool(name="psum", bufs=1, space="PSUM") as pp,
        ):
            # ---- constants into SBUF ----
            t_wlin = cp.tile([128, 4 * H], f32, name="t_wlin")
            t_wg = cp.tile([128, 4 * H], f32, name="t_wg")
            t_wm = cp.tile([128, 4 * H], f32, name="t_wm")
            t_whh = cp.tile([128, 4 * H3], f32, name="t_whh")
            t_wih = cp.tile([8, H3], f32, name="t_wih")
            t_wv = cp.tile([128, 4 * C], f32, name="t_wv")
            t_w12 = cp.tile([128, 8], f32, name="t_w12")
            t_xT = cp.tile([8, S * BC], f32, name="t_xT")
            t_s4 = cp.tile([128, 15 * 256], f32, name="t_s4")
            t_zT4 = cp.tile([128, 256], f32, name="t_zT4")
            t_eye = cp.tile([64, 64], f32, name="t_eye")
            t_mask = cp.tile([64, 256], f32, name="t_mask")
            t_bias = cp.tile([1, BIAS_LEN], f32, name="t_bias")
            t_c15 = cp.tile([64, H], f32, name="t_c15")
            t_nbe = cp.tile([64, 1], f32, name="t_nbe")
            t_enc = cp.tile([BC, S * C], f32, name="t_enc")
            t_d1 = cp.tile([64, 16], f32, name="t_d1")
            t_d2 = cp.tile([64, 16], f32, name="t_d2")
            t_ed = cp.tile([64, 256], f32, name="t_ed")
            t_bg = cp.tile([64, H], f32, name="t_bg")
            t_bm = cp.tile([64, H], f32, name="t_bm")
            t_bet = cp.tile([64, 1], f32, name="t_bet")
            # persistent transposed-activation buffers (row 117 of slice 3 = ones)
            t_gsA = cp.tile([128, 256], f32, name="t_gsA")
            t_gsB = cp.tile([128, 256], f32, name="t_gsB")
            t_hT = cp.tile([128, 256], f32, name="t_hT")
            t_aT = cp.tile([128, 256], f32, name="t_aT")

            nc.sync.dma_start(out=t_wlin[:, :], in_=d_wlin.ap())
            nc.sync.dma_start(out=t_wg[:, :], in_=d_wg.ap())
            nc.sync.dma_start(out=t_wm[:, :], in_=d_wm.ap())
            nc.sync.dma_start(out=t_whh[:, :], in_=d_whh.ap())
            nc.sync.dma_start(out=t_wih[:, :], in_=d_wih.ap())
            nc.sync.dma_start(out=t_wv[:, :], in_=d_wv.ap())
            nc.sync.dma_start(out=t_w12[:, :], in_=d_w12.ap())
            nc.sync.dma_start(out=t_xT[:, :], in_=d_xT.ap())
            nc.sync.dma_start(out=t_s4[:, :], in_=d_s4.ap())
            nc.sync.dma_start(out=t_zT4[:, :], in_=d_zT4.ap())
            nc.sync.dma_start(out=t_eye[:, :], in_=d_eye.ap())
            nc.sync.dma_start(out=t_mask[:, :], in_=d_mask.ap())
            nc.sync.dma_start(out=t_bias[:, :], in_=d_bias.ap())
            nc.sync.dma_start(out=t_bg[:, :], in_=bcast(d_bias, BO_GATE, H, 64))
            nc.sync.dma_start(out=t_bm[:, :], in_=bcast(d_bias, BO_MAP, H, 64))
            nc.sync.dma_start(out=t_bet[:, :], in_=bcast(d_bias, BO_BE, 1, 64))
            for t in (t_gsA, t_gsB, t_hT, t_aT):
                nc.sync.dma_start(out=t[117:118, 192:256], in_=d_ones.ap())

            nc.vector.memset(t_d1[:, :], 0.0)
            nc.vector.memset(t_d2[:, :], 0.0)

            # C15 = 15*sigmoid(b_gate)*b_map  (broadcast over 64 partitions)
            sg0 = wp.tile([64, H], f32, tag="sg")
            nc.scalar.activation(out=sg0[:, :], in_=t_bg[:, :], func=Act.Sigmoid)
            nc.vector.tensor_tensor(out=t_c15[:, :], in0=sg0[:, :], in1=t_bm[:, :], op=Alu.mult)
            nc.vector.tensor_scalar(out=t_c15[:, :], in0=t_c15[:, :],
                                    scalar1=float(S - 1), scalar2=None, op0=Alu.mult)
            # nbe = -b_edge - 1e-7 (threshold incl. the f32 sigmoid-rounding window)
            nc.vector.tensor_scalar(out=t_nbe[:, :], in0=t_bet[:, :],
                                    scalar1=-1.0, scalar2=-1e-7, op0=Alu.mult, op1=Alu.add)

            def mm_group(psum_ap, pairs):
                for i, (l, r) in enumerate(pairs):
                    nc.tensor.matmul(psum_ap, l, r,
                                     start=(i == 0), stop=(i == len(pairs) - 1))

            def transpose_into(psum_t, src, dst):
                # dst: (128,256) persistent sbuf; writes rows 0:117 of slice3 only
                for k in range(4):
                    nc.tensor.transpose(psum_t[0:KT[k], 64 * k:64 * k + 64],
                                        src[:, KOFF[k]:KOFF[k] + KT[k]], t_eye[:, :])
                nc.vector.tensor_copy(out=dst[0:128, 0:192], in_=psum_t[0:128, 0:192])
                nc.vector.tensor_copy(out=dst[0:117, 192:256], in_=psum_t[0:117, 192:256])

            def lhs_sl(buf, k):
                return buf[0:KTL[k], 64 * k:64 * k + 64]

            def hh_pairs(hT, g):
                return [(lhs_sl(hT, k),
                         t_whh[0:KTL[k], H3 * k + H * g:H3 * k + H * g + H]) for k in range(4)]

            def w_pairs(buf, t_w, n):
                return [(lhs_sl(buf, k), t_w[0:KTL[k], n * k:n * k + n]) for k in range(4)]

            gsT_prev = None
            for idx in range(S):
                if idx == 0:
                    pg = pp.tile([64, H], f32, tag="pg")
                    mm_group(pg[:, :], [(t_zT4[0:KTL[k], 64 * k:64 * k + 64],
                                         t_wlin[0:KTL[k], H * k:H * k + H]) for k in range(4)])
                    h_in = wp.tile([64, H], f32, tag="h_in")
                    nc.vector.tensor_copy(out=h_in[:, :], in_=pg[:, :])
                    pt = pp.tile([128, 256], f32, tag="pT")
                    gsT = t_gsA
                    transpose_into(pt, h_in, gsT)
                    hT = gsT  # h_in == gs at idx 0
                else:
                    gsT = t_gsA if (idx % 2 == 0) else t_gsB
                    # a^T = gsT_prev * s  (s = dep[:,idx,idx-1] broadcast, host-prepared)
                    s0 = 256 * (idx - 1)
                    nc.vector.tensor_tensor(out=t_aT[0:128, 0:192], in0=gsT_prev[0:128, 0:192],
                                            in1=t_s4[0:128, s0:s0 + 192], op=Alu.mult)
                    nc.vector.tensor_tensor(out=t_aT[0:117, 192:256], in0=gsT_prev[0:117, 192:256],
                                            in1=t_s4[0:117, s0 + 192:s0 + 256], op=Alu.mult)
                    pg = pp.tile([64, H], f32, tag="pg")
                    pm = pp.tile([64, H], f32, tag="pm")
                    mm_group(pg[:, :], w_pairs(t_aT, t_wg, H))
                    mm_group(pm[:, :], w_pairs(t_aT, t_wm, H))
                    sg = wp.tile([64, H], f32, tag="sg")
                    nc.scalar.activation(out=sg[:, :], in_=pg[:, :], func=Act.Sigmoid)
                    h_in = wp.tile([64, H], f32, tag="h_in")
                    nc.vector.tensor_tensor(out=h_in[:, :], in0=sg[:, :], in1=pm[:, :], op=Alu.mult)
                    nc.vector.tensor_tensor(out=h_in[:, :], in0=h_in[:, :], in1=t_c15[:, :], op=Alu.add)
                    pt = pp.tile([128, 256], f32, tag="pT")
                    hT = t_hT
                    transpose_into(pt, h_in, hT)

                # ---- GRU ----
                xsl = t_xT[0:8, BC * idx:BC * idx + 64]
                pr = pp.tile([64, H], f32, tag="pr")
                pu = pp.tile([64, H], f32, tag="pu")
                phn = pp.tile([64, H], f32, tag="phn")
                pin = pp.tile([64, H], f32, tag="pin")
                mm_group(pr[:, :], hh_pairs(hT, 0) + [(xsl, t_wih[0:8, 0:H])])
                mm_group(phn[:, :], hh_pairs(hT, 2))
                mm_group(pin[:, :], [(xsl, t_wih[0:8, 2 * H:3 * H])])
                mm_group(pu[:, :], hh_pairs(hT, 1) + [(xsl, t_wih[0:8, H:2 * H])])

                r = wp.tile([64, H], f32, tag="r")
                u = wp.tile([64, H], f32, tag="u")
                u1m = wp.tile([64, H], f32, tag="u1m")
                nc.scalar.activation(out=r[:, :], in_=pr[:, :], func=Act.Sigmoid)
                t1 = wp.tile([64, H], f32, tag="t1")
                nc.vector.tensor_tensor(out=t1[:, :], in0=r[:, :], in1=phn[:, :], op=Alu.mult)
                t2 = wp.tile([64, H], f32, tag="t2")
                nc.vector.tensor_tensor(out=t2[:, :], in0=t1[:, :], in1=pin[:, :], op=Alu.add)
                n = wp.tile([64, H], f32, tag="n")
                nc.scalar.activation(out=n[:, :], in_=t2[:, :], func=Act.Tanh)
                nc.scalar.activation(out=u[:, :], in_=pu[:, :], func=Act.Sigmoid)
                nc.scalar.activation(out=u1m[:, :], in_=pu[:, :], func=Act.Sigmoid, scale=-1.0)
                w = wp.tile([64, H], f32, tag="w")
                nc.vector.tensor_tensor(out=w[:, :], in0=u[:, :], in1=h_in[:, :], op=Alu.mult)
                m1 = wp.tile([64, H], f32, tag="m1")
                nc.vector.tensor_tensor(out=m1[:, :], in0=u1m[:, :], in1=n[:, :], op=Alu.mult)
                hv = wp.tile([64, H], f32, tag="hv")
                nc.vector.tensor_tensor(out=hv[:, :], in0=m1[:, :], in1=w[:, :], op=Alu.add)

                # ---- transpose hv -> gsT (for next step / dots / next enc) ----
                pt2 = pp.tile([128, 256], f32, tag="pT")
                gsT_new = t_gsB if (idx % 2 == 0) else t_gsA
                transpose_into(pt2, hv, gsT_new)

                # ---- dots d1,d2 ----
                psm = pp.tile([64, 16], f32, tag="psml")
                mm_group(psm[:, 8:10], w_pairs(gsT_new, t_w12, 2))
                nc.vector.tensor_copy(out=t_d1[:, idx:idx + 1], in_=psm[:, 8:9])
                nc.vector.tensor_copy(out=t_d2[:, idx:idx + 1], in_=psm[:, 9:10])

                # ---- enc softmax for this idx (from gs = hv_{idx-1}) ----
                enc_src = gsT if idx == 0 else gsT_prev
                plog = pp.tile([64, 16], f32, tag="psml")
                mm_group(plog[:, 0:C], w_pairs(enc_src, t_wv, C))
                mx = wp.tile([64, 1], f32, tag="mx")
                nc.vector.tensor_reduce(out=mx[:, :], in_=plog[:, 0:C], axis=Axis.X, op=Alu.max)
                nc.vector.tensor_scalar(out=mx[:, :], in0=mx[:, :],
                                        scalar1=-0.5, scalar2=None, op0=Alu.mult)
                th = wp.tile([64, C], f32, tag="th")
                nc.scalar.activation(out=th[:, :], in_=plog[:, 0:C], func=Act.Tanh,
                                     bias=mx[:, :], scale=0.5)
                num = wp.tile([64, C], f32, tag="num")
                nc.vector.tensor_scalar(out=num[:, :], in0=th[:, :],
                                        scalar1=1.0, scalar2=None, op0=Alu.add)
                den = wp.tile([64, C], f32, tag="den")
                nc.vector.tensor_scalar(out=den[:, :], in0=th[:, :],
                                        scalar1=-1.0, scalar2=1.0, op0=Alu.mult, op1=Alu.add)
                rd = wp.tile([64, C], f32, tag="rd")
                nc.vector.reciprocal(out=rd[:, :], in_=den[:, :])
                ex = wp.tile([64, C], f32, tag="ex")
                nc.vector.tensor_tensor(out=ex[:, :], in0=num[:, :], in1=rd[:, :], op=Alu.mult)
                sm = wp.tile([64, 1], f32, tag="sm")
                nc.vector.tensor_reduce(out=sm[:, :], in_=ex[:, :], axis=Axis.X, op=Alu.add)
                rc = wp.tile([64, 1], f32, tag="rc")
                nc.vector.reciprocal(out=rc[:, :], in_=sm[:, :])
                nc.vector.tensor_scalar(out=t_enc[:, C * idx:C * idx + C], in0=ex[:, :],
                                        scalar1=rc[:, :], scalar2=None, op0=Alu.mult)

                # ---- edge-margin row idx: ED[:, 16*idx + j] = d1[idx] + d2[j] ----
                nc.vector.tensor_scalar(out=t_ed[:, 16 * idx:16 * idx + 16], in0=t_d2[:, :],
                                        scalar1=t_d1[:, idx:idx + 1], scalar2=None, op0=Alu.add)

                gsT_prev = gsT_new

            # ---- finalize edges ----
            tth = cp.tile([64, 256], f32, name="tth")
            nc.vector.tensor_scalar(out=tth[:, :], in0=t_ed[:, :],
                                    scalar1=t_nbe[:, :], scalar2=None, op0=Alu.is_ge)
            nc.vector.tensor_tensor(out=tth[:, :], in0=tth[:, :], in1=t_mask[:, :], op=Alu.mult)
            sd = cp.tile([64, 16], f32, name="sd")
            nc.vector.tensor_tensor(out=sd[:, :], in0=t_d1[:, :], in1=t_d2[:, :], op=Alu.add)
            sdt = cp.tile([64, 16], f32, name="sdt")
            nc.vector.tensor_scalar(out=sdt[:, :], in0=sd[:, :],
                                    scalar1=t_nbe[:, :], scalar2=None, op0=Alu.is_ge)
            for i in range(1, 16):
                nc.vector.tensor_copy(out=tth[:, 17 * i - 1:17 * i], in_=sdt[:, i - 1:i])

            nc.sync.dma_start(out=d_odep.ap(), in_=tth[:, :])
            nc.sync.dma_start(out=d_oenc.ap(), in_=t_enc[:, :])

    nc.compile()
    return nc


def _host_prep(z, dep_graph, node_encoding, W_lin1, b_lin1, W_vert, b_vert,
               W_edge, b_edge, W_gate, b_gate, W_map, b_map, W_ih, b_ih, W_hh, b_hh):
    f = np.float32

    def ktiled(WT, bias=None):  # WT: (H, N) -> (128, 4N); bias lands at row 117 of tile 3
        N = WT.shape[1]
        out = np.zeros((128, 4 * N), f)
        for k in range(4):
            out[0:KT[k], N * k:N * (k + 1)] = WT[KOFF[k]:KOFF[k] + KT[k], :]
        if bias is not None:
            out[117, 3 * N:3 * N + N] = bias
        return out

    whhT = np.zeros((128, 4 * H3), f)
    whhsrc = W_hh.T.astype(f)  # (H, 3H)
    for k in range(4):
        whhT[0:KT[k], H3 * k:H3 * (k + 1)] = whhsrc[KOFF[k]:KOFF[k] + KT[k], :]
    whhT[117, 3 * H3:4 * H3] = b_hh

    shared = {
        "WlinT": ktiled(W_lin1.T.astype(f), b_lin1),
        "WgT": ktiled(W_gate.T.astype(f), b_gate),
        "WmT": ktiled(W_map.T.astype(f), b_map),
        "WhhT": whhT,
        "WvT": ktiled(W_vert.T.astype(f), b_vert),
        "w12": ktiled(W_edge.reshape(2, H).T.astype(f)),
        "eye64": np.eye(64, dtype=f),
        "onesrow": np.ones((1, 64), f),
    }
    wih = np.zeros((8, H3), f)
    wih[0:7] = W_ih.T
    wih[7] = b_ih
    shared["WihT"] = wih
    mask = np.zeros((64, 256), f)
    for i in range(16):
        for j in range(16):
            if j <= i - 2:
                mask[:, 16 * i + j] = 1.0
    shared["maskOD"] = mask
    bias = np.zeros((1, BIAS_LEN), f)
    bias[0, BO_GATE:BO_GATE + H] = b_gate
    bias[0, BO_MAP:BO_MAP + H] = b_map
    bias[0, BO_BE] = np.asarray(b_edge).ravel()[0]
    shared["BIASROW"] = bias

    in_maps = []
    for c in range(NCORES):
        sl = slice(c * BC, (c + 1) * BC)
        zc = np.asarray(z[sl], f)
        zt4 = np.zeros((128, 256), f)
        for k in range(4):
            zt4[0:KT[k], 64 * k:64 * k + 64] = zc[:, KOFF[k]:KOFF[k] + KT[k]].T
        zt4[117, 192:256] = 1.0  # ones row for b_lin1
        xt = np.zeros((8, S * BC), f)
        nec = np.asarray(node_encoding[sl], f)  # (64, S, C)
        for idx in range(S):
            xt[0:7, BC * idx:BC * (idx + 1)] = nec[:, idx, :].T
        xt[7] = 1.0
        depc = np.asarray(dep_graph[sl], f)
        s4 = np.zeros((128, 15 * 256), f)
        for i in range(1, S):
            sub = depc[:, i, i - 1]  # (64,)
            s4[:, 256 * (i - 1):256 * i] = np.tile(sub[None, :], (128, 4))
        m = dict(shared)
        m.update({"zT4": zt4, "xT": xt, "S4r": s4})
        in_maps.append(m)
    return in_maps


def kernel(**inputs):
    from concourse.bass_utils import run_bass_kernel_spmd

    if "nc" not in _CACHE:
        _CACHE["nc"] = _build_module()
    nc = _CACHE["nc"]
    in_maps = _host_prep(**inputs)
    res = run_bass_kernel_spmd(nc, in_maps, core_ids=list(range(NCORES)))
    dep_out = np.concatenate(
        [res.results[c]["out_dep"].reshape(BC, S, S) for c in range(NCORES)], axis=0)
    enc_out = np.concatenate(
        [res.results[c]["out_enc"].reshape(BC, S, C) for c in range(NCORES)], axis=0)
    return dep_out.astype(np.float32), enc_out.astype(np.float32)


# revision 17
# speedup vs baseline: 1.0571x; 1.0571x over previous
"""Trainium2 Bass kernel for nn_Decoder (gnn_message_passing).

Mathematical simplification of the reference (verified exact vs the jax oracle):
the reference's inner scan collapses — only the immediate predecessor (idx-1)
contributes to message aggregation, hv_new is invariant across inner steps, and
edge decisions reduce to per-node dot products d1[j]=hv_j.w1, d2[j]=hv_j.w2
thresholded at sigmoid>=0.5.

Per outer step idx (batch-on-partitions layout, 64 batch rows/core):
  enc[idx] = softmax(gs @ Wvert.T + bvert)          (gs = hv_{idx-1}, gs0 = z@Wlin1.T+blin1)
  a        = dep[:,idx,idx-1] * hv_{idx-1}           (idx>=1)
  h_in     = 15*sigmoid(b_gate)*b_map + sigmoid(a@Wg.T+bg) * (a@Wm.T+bm)   (h_in=gs0 at idx=0)
  gru gates from h_in and x_idx -> hv_idx
  d1[idx] = hv.w1, d2[idx] = hv.w2
edges[i,j] = step(d1[i]+d2[j]+be) for j<=i-2; edges[i,i-1] = step(d1[i-1]+d2[i-1]+be).

All matmuls run in fp32 (fp32r is an 11-bit-mantissa format — too coarse for the
hard edge thresholds). Biases enter via a constant ones-row appended to the
stationary operand (row 117 of the K-tile-3 slice) and bias rows baked into the
weight layouts, so no separate bias matmuls are needed.

Sharding: pure data parallel, batch 512 -> 64 per core across 8 cores.
"""

import numpy as np

B, S, H, C = 512, 16, 501, 7
NCORES = 8
BC = B // NCORES  # 64 batch rows per core
KT = [128, 128, 128, 117]    # K tiles over H=501 (data rows)
KTL = [128, 128, 128, 125]   # lhsT/rhs rows (tile 3: 117 data + ones row + 7 x rows)
KOFF = [0, 128, 256, 384]
H3 = 3 * H  # 1503

# packed bias/constant row: b_gate | b_map | b_edge | b_ih[0:2H] | b_hh[0:2H] | b_ih_n
BO_GATE, BO_MAP, BO_BE = 0, H, 2 * H
BO_IHRU = 2 * H + 1
BO_HHRU = BO_IHRU + 2 * H
BO_IHN = BO_HHRU + 2 * H
BIAS_LEN = BO_IHN + H

_CACHE = {}


def _build_module():
    import concourse.bass as bass
    import concourse.bacc as bacc
    import concourse.mybir as mybir
    from concourse.tile import TileContext

    f32 = mybir.dt.float32
    Alu = mybir.AluOpType
    Act = mybir.ActivationFunctionType
    Axis = mybir.AxisListType

    nc = bacc.Bacc("TRN2", target_bir_lowering=False, debug=False,
                   enable_asserts=False, num_devices=NCORES)

    # ---- DRAM I/O ----
    d_zT4 = nc.dram_tensor("zT4", [128, 256], f32, kind="ExternalInput")
    d_wlin = nc.dram_tensor("WlinT", [128, 4 * H], f32, kind="ExternalInput")
    d_wg = nc.dram_tensor("WgT", [128, 4 * H], f32, kind="ExternalInput")
    d_wm = nc.dram_tensor("WmT", [128, 4 * H], f32, kind="ExternalInput")
    d_whh = nc.dram_tensor("WhhT", [128, 4 * H3], f32, kind="ExternalInput")
    d_wih = nc.dram_tensor("WihT", [8, H3], f32, kind="ExternalInput")
    d_wv = nc.dram_tensor("WvT", [128, 4 * C], f32, kind="ExternalInput")
    d_w12 = nc.dram_tensor("w12", [128, 8], f32, kind="ExternalInput")
    d_xT = nc.dram_tensor("xT", [8, S * BC], f32, kind="ExternalInput")
    d_s4 = nc.dram_tensor("S4r", [128, 15 * 256], f32, kind="ExternalInput")
    d_eye = nc.dram_tensor("eye64", [64, 64], f32, kind="ExternalInput")
    d_mask = nc.dram_tensor("maskOD", [64, 256], f32, kind="ExternalInput")
    d_bias = nc.dram_tensor("BIASROW", [1, BIAS_LEN], f32, kind="ExternalInput")
    d_ones = nc.dram_tensor("onesrow", [1, 64], f32, kind="ExternalInput")
    d_zero8 = nc.dram_tensor("zero8", [8, 64], f32, kind="ExternalInput")
    d_cls = nc.dram_tensor("cls", [64, 16], mybir.dt.int32, kind="ExternalInput")
    d_odep = nc.dram_tensor("out_dep", [BC, 256], f32, kind="ExternalOutput")
    d_oenc = nc.dram_tensor("out_enc", [BC, S * C], f32, kind="ExternalOutput")

    def bcast(dram_handle, col0, ncols, nparts):
        ap = dram_handle.ap()
        return bass.AP(tensor=ap.tensor, offset=ap.offset + col0,
                       ap=[[0, nparts], [1, ncols]])

    with TileContext(nc) as tc:
        with (
            tc.tile_pool(name="const", bufs=1) as cp,
            tc.tile_pool(name="work", bufs=2) as wp,
            tc.tile_pool(name="psum", bufs=1, space="PSUM") as pp,
            tc.tile_pool(name="dram", bufs=1, space="DRAM") as dp,
        ):
            # ---- constants into SBUF ----
            t_wlin = cp.tile([128, 4 * H], f32, name="t_wlin")
            t_wg = cp.tile([128, 4 * H], f32, name="t_wg")
            t_wm = cp.tile([128, 4 * H], f32, name="t_wm")
            t_whh = cp.tile([128, 4 * H3], f32, name="t_whh")
            t_wih = cp.tile([8, H3], f32, name="t_wih")
            t_wv = cp.tile([128, 4 * C], f32, name="t_wv")
            t_w12 = cp.tile([128, 8], f32, name="t_w12")
            t_xT = cp.tile([8, S * BC], f32, name="t_xT")
            t_s4 = cp.tile([128, 15 * 256], f32, name="t_s4")
            t_zT4 = cp.tile([128, 256], f32, name="t_zT4")
            t_eye = cp.tile([64, 64], f32, name="t_eye")
            t_mask = cp.tile([64, 256], f32, name="t_mask")
            t_bias = cp.tile([1, BIAS_LEN], f32, name="t_bias")
            t_c15 = cp.tile([64, H], f32, name="t_c15")
            t_nbe = cp.tile([64, 1], f32, name="t_nbe")
            t_enc = cp.tile([BC, S * C], f32, name="t_enc")
            t_d1 = cp.tile([64, 16], f32, name="t_d1")
            t_d2 = cp.tile([64, 16], f32, name="t_d2")
            t_ed = cp.tile([64, 256], f32, name="t_ed")
            t_bg = cp.tile([64, H], f32, name="t_bg")
            t_bm = cp.tile([64, H], f32, name="t_bm")
            t_bet = cp.tile([64, 1], f32, name="t_bet")
            t_cls = cp.tile([64, 16], mybir.dt.int32, name="t_cls")
            t_gin = cp.tile([64, S * H], f32, name="t_gin")
            t_bin = cp.tile([7, H], f32, name="t_bin")
            t_ginw = cp.tile([7, H], f32, name="t_ginw")
            t_bsc = cp.tile([1, 2 * H], f32, name="t_bsc")
            # persistent transposed-activation buffers (row 117 of slice 3 = ones)
            t_gsA = cp.tile([128, 256], f32, name="t_gsA")
            t_gsB = cp.tile([128, 256], f32, name="t_gsB")
            t_hT = cp.tile([128, 256], f32, name="t_hT")
            t_aT = cp.tile([128, 256], f32, name="t_aT")

            nc.sync.dma_start(out=t_wlin[:, :], in_=d_wlin.ap())
            nc.sync.dma_start(out=t_wg[:, :], in_=d_wg.ap())
            nc.sync.dma_start(out=t_wm[:, :], in_=d_wm.ap())
            nc.sync.dma_start(out=t_whh[:, :], in_=d_whh.ap())
            nc.sync.dma_start(out=t_wih[:, :], in_=d_wih.ap())
            nc.sync.dma_start(out=t_wv[:, :], in_=d_wv.ap())
            nc.sync.dma_start(out=t_w12[:, :], in_=d_w12.ap())
            nc.sync.dma_start(out=t_xT[:, :], in_=d_xT.ap())
            nc.sync.dma_start(out=t_s4[:, :], in_=d_s4.ap())
            nc.sync.dma_start(out=t_zT4[:, :], in_=d_zT4.ap())
            nc.sync.dma_start(out=t_eye[:, :], in_=d_eye.ap())
            nc.sync.dma_start(out=t_mask[:, :], in_=d_mask.ap())
            nc.sync.dma_start(out=t_bias[:, :], in_=d_bias.ap())
            nc.sync.dma_start(out=t_bg[:, :], in_=bcast(d_bias, BO_GATE, H, 64))
            nc.sync.dma_start(out=t_bm[:, :], in_=bcast(d_bias, BO_MAP, H, 64))
            nc.sync.dma_start(out=t_bet[:, :], in_=bcast(d_bias, BO_BE, 1, 64))
            for t in (t_gsA, t_gsB, t_hT, t_aT):
                nc.sync.dma_start(out=t[117:118, 192:256], in_=d_ones.ap())
            for t in (t_gsB, t_aT):
                nc.sync.dma_start(out=t[118:126, 192:256], in_=d_zero8.ap())
            # gsA doubles as hT at idx 0: rows 118:125 carry x_0
            nc.sync.dma_start(out=t_gsA[118:125, 192:256], in_=d_xT.ap()[0:7, 0:64])
            nc.sync.dma_start(out=t_gsA[125:126, 192:256], in_=d_zero8.ap()[0:1, :])
            nc.sync.dma_start(out=t_cls[:, :], in_=d_cls.ap())

            nc.vector.memset(t_d1[:, :], 0.0)
            nc.vector.memset(t_d2[:, :], 0.0)

            # fold b_ih(r,u)+b_hh(r,u) into the hh-weight bias row (row 117 of K-tile 3)
            nc.vector.tensor_tensor(out=t_bsc[0:1, :], in0=t_bias[0:1, BO_IHRU:BO_IHRU + 2 * H],
                                    in1=t_bias[0:1, BO_HHRU:BO_HHRU + 2 * H], op=Alu.add)
            nc.sync.dma_start(out=t_whh[117:118, 3 * H3:3 * H3 + 2 * H], in_=t_bsc[0:1, :])

            # i_n lookup table: W_ih_n.T rows + b_ih_n, staged to DRAM then gathered per idx
            nc.sync.dma_start(out=t_bin[:, :], in_=bcast(d_bias, BO_IHN, H, 7))
            nc.vector.tensor_tensor(out=t_ginw[:, :], in0=t_wih[0:7, 2 * H:3 * H],
                                    in1=t_bin[:, :], op=Alu.add)

            # C15 = 15*sigmoid(b_gate)*b_map  (broadcast over 64 partitions)
            sg0 = wp.tile([64, H], f32, tag="sg")
            nc.scalar.activation(out=sg0[:, :], in_=t_bg[:, :], func=Act.Sigmoid)
            nc.vector.tensor_tensor(out=t_c15[:, :], in0=sg0[:, :], in1=t_bm[:, :], op=Alu.mult)
            nc.vector.tensor_scalar(out=t_c15[:, :], in0=t_c15[:, :],
                                    scalar1=float(S - 1), scalar2=None, op0=Alu.mult)
            # nbe = -b_edge - 1e-7 (threshold incl. the f32 sigmoid-rounding window)
            nc.vector.tensor_scalar(out=t_nbe[:, :], in0=t_bet[:, :],
                                    scalar1=-1.0, scalar2=-1e-7, op0=Alu.mult, op1=Alu.add)

            def mm_group(psum_ap, pairs):
                for i, (l, r) in enumerate(pairs):
                    nc.tensor.matmul(psum_ap, l, r,
                                     start=(i == 0), stop=(i == len(pairs) - 1))

            def transpose_into(psum_t, src, dst):
                # dst: (128,256) persistent sbuf; writes rows 0:117 of slice3 only
                for k in range(4):
                    nc.tensor.transpose(psum_t[0:KT[k], 64 * k:64 * k + 64],
                                        src[:, KOFF[k]:KOFF[k] + KT[k]], t_eye[:, :])
                nc.vector.tensor_copy(out=dst[0:128, 0:192], in_=psum_t[0:128, 0:192])
                nc.vector.tensor_copy(out=dst[0:117, 192:256], in_=psum_t[0:117, 192:256])

            def lhs_sl(buf, k):
                return buf[0:KTL[k], 64 * k:64 * k + 64]

            def hh_pairs(hT, g):
                return [(lhs_sl(hT, k),
                         t_whh[0:KTL[k], H3 * k + H * g:H3 * k + H * g + H]) for k in range(4)]

            def w_pairs(buf, t_w, n):
                return [(lhs_sl(buf, k), t_w[0:KTL[k], n * k:n * k + n]) for k in range(4)]

            gsT_prev = None
            for idx in range(S):
                if idx == 0:
                    pgm = pp.tile([128, 512], f32, tag="pgm")
                    mm_group(pgm[0:64, 0:H], [(t_zT4[0:KTL[k], 64 * k:64 * k + 64],
                                               t_wlin[0:KTL[k], H * k:H * k + H]) for k in range(4)])
                    h_in = wp.tile([64, H], f32, tag="h_in")
                    nc.vector.tensor_copy(out=h_in[:, :], in_=pgm[0:64, 0:H])
                    pt = pp.tile([128, 256], f32, tag="pT")
                    gsT = t_gsA
                    transpose_into(pt, h_in, gsT)
                    hT = gsT  # h_in == gs at idx 0
                else:
                    gsT = t_gsA if (idx % 2 == 0) else t_gsB
                    # a^T = gsT_prev * s  (s = dep[:,idx,idx-1] broadcast, host-prepared)
                    s0 = 256 * (idx - 1)
                    nc.vector.tensor_tensor(out=t_aT[0:128, 0:192], in0=gsT_prev[0:128, 0:192],
                                            in1=t_s4[0:128, s0:s0 + 192], op=Alu.mult)
                    nc.vector.tensor_tensor(out=t_aT[0:117, 192:256], in0=gsT_prev[0:117, 192:256],
                                            in1=t_s4[0:117, s0 + 192:s0 + 256], op=Alu.mult)
                    pgm = pp.tile([128, 512], f32, tag="pgm")
                    gp = w_pairs(t_aT, t_wg, H)
                    mp = w_pairs(t_aT, t_wm, H)
                    for k in range(4):  # interleave: col-groups 0:64 / 64:128 run concurrently
                        nc.tensor.matmul(pgm[0:64, 0:H], gp[k][0], gp[k][1],
                                         start=(k == 0), stop=(k == 3),
                                         skip_group_check=True)
                        nc.tensor.matmul(pgm[64:128, 0:H], mp[k][0], mp[k][1],
                                         start=(k == 0), stop=(k == 3),
                                         skip_group_check=True)
                    sg = wp.tile([64, H], f32, tag="sg")
                    nc.scalar.activation(out=sg[:, :], in_=pgm[0:64, 0:H], func=Act.Sigmoid)
                    h_in = wp.tile([64, H], f32, tag="h_in")
                    nc.vector.tensor_tensor(out=h_in[:, :], in0=sg[:, :], in1=pgm[64:128, 0:H], op=Alu.mult)
                    nc.vector.tensor_tensor(out=h_in[:, :], in0=h_in[:, :], in1=t_c15[:, :], op=Alu.add)
                    pt = pp.tile([128, 256], f32, tag="pT")
                    hT = t_hT
                    transpose_into(pt, h_in, hT)

                # ---- GRU ----
                if idx > 0:
                    # stage x_idx into the spare K-tile-3 rows of hT (r/u gi comes free)
                    nc.sync.dma_start(out=t_hT[118:125, 192:256],
                                      in_=t_xT[0:7, BC * idx:BC * idx + 64])
                # pair [r || hn] first (both feed the tanh chain); u after (needed late)
                prh = pp.tile([128, 512], f32, tag="prh")
                rp = hh_pairs(hT, 0)
                hp = hh_pairs(hT, 2)
                for k in range(4):
                    nc.tensor.matmul(prh[0:64, 0:H], rp[k][0], rp[k][1],
                                     start=(k == 0), stop=(k == 3),
                                     skip_group_check=True)
                    nc.tensor.matmul(prh[64:128, 0:H], hp[k][0], hp[k][1],
                                     start=(k == 0), stop=(k == 3),
                                     skip_group_check=True)
                pui = pp.tile([64, 512], f32, tag="pui")
                mm_group(pui[0:64, 0:H], hh_pairs(hT, 1))
                pin = pp.tile([64, 512], f32, tag="pin")
                nc.tensor.matmul(pin[0:64, 0:H], t_xT[0:8, BC * idx:BC * idx + 64],
                                 t_wih[0:8, 2 * H:3 * H], start=True, stop=True)

                r = wp.tile([64, H], f32, tag="r")
                u = wp.tile([64, H], f32, tag="u")
                u1m = wp.tile([64, H], f32, tag="u1m")
                nc.scalar.activation(out=r[:, :], in_=prh[0:64, 0:H], func=Act.Sigmoid)
                t1 = wp.tile([64, H], f32, tag="t1")
                nc.vector.tensor_tensor(out=t1[:, :], in0=r[:, :], in1=prh[64:128, 0:H], op=Alu.mult)
                t2 = wp.tile([64, H], f32, tag="t2")
                nc.vector.tensor_tensor(out=t2[:, :], in0=t1[:, :],
                                        in1=pin[0:64, 0:H], op=Alu.add)
                n = wp.tile([64, H], f32, tag="n")
                nc.scalar.activation(out=n[:, :], in_=t2[:, :], func=Act.Tanh)
                nc.scalar.activation(out=u[:, :], in_=pui[0:64, 0:H], func=Act.Sigmoid)
                nc.scalar.activation(out=u1m[:, :], in_=pui[0:64, 0:H], func=Act.Sigmoid, scale=-1.0)
                w = wp.tile([64, H], f32, tag="w")
                nc.vector.tensor_tensor(out=w[:, :], in0=u[:, :], in1=h_in[:, :], op=Alu.mult)
                m1 = wp.tile([64, H], f32, tag="m1")
                nc.vector.tensor_tensor(out=m1[:, :], in0=u1m[:, :], in1=n[:, :], op=Alu.mult)
                hv = wp.tile([64, H], f32, tag="hv")
                nc.vector.tensor_tensor(out=hv[:, :], in0=m1[:, :], in1=w[:, :], op=Alu.add)

                # ---- transpose hv -> gsT (for next step / dots / next enc) ----
                pt2 = pp.tile([128, 256], f32, tag="pT")
                gsT_new = t_gsB if (idx % 2 == 0) else t_gsA
                transpose_into(pt2, hv, gsT_new)

                # ---- dots d1,d2 ----
                psm = pp.tile([64, 16], f32, tag="psml")
                mm_group(psm[:, 8:10], w_pairs(gsT_new, t_w12, 2))
                nc.vector.tensor_copy(out=t_d1[:, idx:idx + 1], in_=psm[:, 8:9])
                nc.vector.tensor_copy(out=t_d2[:, idx:idx + 1], in_=psm[:, 9:10])

                # ---- enc softmax for this idx (from gs = hv_{idx-1}) ----
                enc_src = gsT if idx == 0 else gsT_prev
                plog = pp.tile([64, 16], f32, tag="psml")
                mm_group(plog[:, 0:C], w_pairs(enc_src, t_wv, C))
                mx = wp.tile([64, 1], f32, tag="mx")
                nc.vector.tensor_reduce(out=mx[:, :], in_=plog[:, 0:C], axis=Axis.X, op=Alu.max)
                nc.vector.tensor_scalar(out=mx[:, :], in0=mx[:, :],
                                        scalar1=-0.5, scalar2=None, op0=Alu.mult)
                th = wp.tile([64, C], f32, tag="th")
                nc.scalar.activation(out=th[:, :], in_=plog[:, 0:C], func=Act.Tanh,
                                     bias=mx[:, :], scale=0.5)
                num = wp.tile([64, C], f32, tag="num")
                nc.vector.tensor_scalar(out=num[:, :], in0=th[:, :],
                                        scalar1=1.0, scalar2=None, op0=Alu.add)
                den = wp.tile([64, C], f32, tag="den")
                nc.vector.tensor_scalar(out=den[:, :], in0=th[:, :],
                                        scalar1=-1.0, scalar2=1.0, op0=Alu.mult, op1=Alu.add)
                rd = wp.tile([64, C], f32, tag="rd")
                nc.vector.reciprocal(out=rd[:, :], in_=den[:, :])
                ex = wp.tile([64, C], f32, tag="ex")
                nc.vector.tensor_tensor(out=ex[:, :], in0=num[:, :], in1=rd[:, :], op=Alu.mult)
                sm = wp.tile([64, 1], f32, tag="sm")
                nc.vector.tensor_reduce(out=sm[:, :], in_=ex[:, :], axis=Axis.X, op=Alu.add)
                rc = wp.tile([64, 1], f32, tag="rc")
                nc.vector.reciprocal(out=rc[:, :], in_=sm[:, :])
                nc.vector.tensor_scalar(out=t_enc[:, C * idx:C * idx + C], in0=ex[:, :],
                                        scalar1=rc[:, :], scalar2=None, op0=Alu.mult)

                # ---- edge-margin row idx: ED[:, 16*idx + j] = d1[idx] + d2[j] ----
                nc.vector.tensor_scalar(out=t_ed[:, 16 * idx:16 * idx + 16], in0=t_d2[:, :],
                                        scalar1=t_d1[:, idx:idx + 1], scalar2=None, op0=Alu.add)

                gsT_prev = gsT_new

            # ---- finalize edges ----
            tth = cp.tile([64, 256], f32, name="tth")
            nc.vector.tensor_scalar(out=tth[:, :], in0=t_ed[:, :],
                                    scalar1=t_nbe[:, :], scalar2=None, op0=Alu.is_ge)
            nc.vector.tensor_tensor(out=tth[:, :], in0=tth[:, :], in1=t_mask[:, :], op=Alu.mult)
            sd = cp.tile([64, 16], f32, name="sd")
            nc.vector.tensor_tensor(out=sd[:, :], in0=t_d1[:, :], in1=t_d2[:, :], op=Alu.add)
            sdt = cp.tile([64, 16], f32, name="sdt")
            nc.vector.tensor_scalar(out=sdt[:, :], in0=sd[:, :],
                                    scalar1=t_nbe[:, :], scalar2=None, op0=Alu.is_ge)
            for i in range(1, 16):
                nc.vector.tensor_copy(out=tth[:, 17 * i - 1:17 * i], in_=sdt[:, i - 1:i])

            nc.sync.dma_start(out=d_odep.ap(), in_=tth[:, :])
            nc.sync.dma_start(out=d_oenc.ap(), in_=t_enc[:, :])

    nc.compile()
    return nc


def _host_prep(z, dep_graph, node_encoding, W_lin1, b_lin1, W_vert, b_vert,
               W_edge, b_edge, W_gate, b_gate, W_map, b_map, W_ih, b_ih, W_hh, b_hh):
    f = np.float32

    def ktiled(WT, bias=None):  # WT: (H, N) -> (128, 4N); bias lands at row 117 of tile 3
        N = WT.shape[1]
        out = np.zeros((128, 4 * N), f)
        for k in range(4):
            out[0:KT[k], N * k:N * (k + 1)] = WT[KOFF[k]:KOFF[k] + KT[k], :]
        if bias is not None:
            out[117, 3 * N:3 * N + N] = bias
        return out

    whhT = np.zeros((128, 4 * H3), f)
    whhsrc = W_hh.T.astype(f)  # (H, 3H)
    for k in range(4):
        whhT[0:KT[k], H3 * k:H3 * (k + 1)] = whhsrc[KOFF[k]:KOFF[k] + KT[k], :]
    whhT[117, 3 * H3:4 * H3] = b_hh  # r/u thirds get b_ih added on-device
    # spare rows 118:125 of K-tile 3 carry W_ih.T for the r/u thirds (x rows of lhsT)
    whhT[118:125, 3 * H3:3 * H3 + 2 * H] = W_ih.T[:, 0:2 * H]

    shared = {
        "WlinT": ktiled(W_lin1.T.astype(f), b_lin1),
        "WgT": ktiled(W_gate.T.astype(f), b_gate),
        "WmT": ktiled(W_map.T.astype(f), b_map),
        "WhhT": whhT,
        "WvT": ktiled(W_vert.T.astype(f), b_vert),
        "w12": ktiled(W_edge.reshape(2, H).T.astype(f)),
        "eye64": np.eye(64, dtype=f),
        "onesrow": np.ones((1, 64), f),
    }
    wih = np.zeros((8, H3), f)
    wih[0:7] = W_ih.T
    wih[7] = b_ih
    shared["WihT"] = wih
    mask = np.zeros((64, 256), f)
    for i in range(16):
        for j in range(16):
            if j <= i - 2:
                mask[:, 16 * i + j] = 1.0
    shared["maskOD"] = mask
    bias = np.zeros((1, BIAS_LEN), f)
    bias[0, BO_GATE:BO_GATE + H] = b_gate
    bias[0, BO_MAP:BO_MAP + H] = b_map
    bias[0, BO_BE] = np.asarray(b_edge).ravel()[0]
    bias[0, BO_IHRU:BO_IHRU + 2 * H] = b_ih[0:2 * H]
    bias[0, BO_HHRU:BO_HHRU + 2 * H] = b_hh[0:2 * H]
    bias[0, BO_IHN:BO_IHN + H] = b_ih[2 * H:3 * H]
    shared["BIASROW"] = bias
    shared["zero8"] = np.zeros((8, 64), f)

    in_maps = []
    for c in range(NCORES):
        sl = slice(c * BC, (c + 1) * BC)
        zc = np.asarray(z[sl], f)
        zt4 = np.zeros((128, 256), f)
        for k in range(4):
            zt4[0:KT[k], 64 * k:64 * k + 64] = zc[:, KOFF[k]:KOFF[k] + KT[k]].T
        zt4[117, 192:256] = 1.0  # ones row for b_lin1
        xt = np.zeros((8, S * BC), f)
        nec = np.asarray(node_encoding[sl], f)  # (64, S, C)
        for idx in range(S):
            xt[0:7, BC * idx:BC * (idx + 1)] = nec[:, idx, :].T
        xt[7] = 1.0
        cls = np.ascontiguousarray(np.argmax(nec, axis=2).astype(np.int32))  # (64, S)
        depc = np.asarray(dep_graph[sl], f)
        s4 = np.zeros((128, 15 * 256), f)
        for i in range(1, S):
            sub = depc[:, i, i - 1]  # (64,)
            s4[:, 256 * (i - 1):256 * i] = np.tile(sub[None, :], (128, 4))
        m = dict(shared)
        m.update({"zT4": zt4, "xT": xt, "S4r": s4, "cls": cls})
        in_maps.append(m)
    return in_maps


def kernel(**inputs):
    from concourse.bass_utils import run_bass_kernel_spmd

    if "nc" not in _CACHE:
        _CACHE["nc"] = _build_module()
    nc = _CACHE["nc"]
    in_maps = _host_prep(**inputs)
    res = run_bass_kernel_spmd(nc, in_maps, core_ids=list(range(NCORES)))
    dep_out = np.concatenate(
        [res.results[c]["out_dep"].reshape(BC, S, S) for c in range(NCORES)], axis=0)
    enc_out = np.concatenate(
        [res.results[c]["out_enc"].reshape(BC, S, C) for c in range(NCORES)], axis=0)
    return dep_out.astype(np.float32), enc_out.astype(np.float32)


# revision 20
# speedup vs baseline: 1.0977x; 1.0384x over previous
"""Trainium2 Bass kernel for nn_Decoder (gnn_message_passing).

Mathematical simplification of the reference (verified exact vs the jax oracle):
the reference's inner scan collapses — only the immediate predecessor (idx-1)
contributes to message aggregation, hv_new is invariant across inner steps, and
edge decisions reduce to per-node dot products d1[j]=hv_j.w1, d2[j]=hv_j.w2
thresholded at sigmoid>=0.5.

Per outer step idx (batch-on-partitions layout, 64 batch rows/core):
  enc[idx] = softmax(gs @ Wvert.T + bvert)          (gs = hv_{idx-1}, gs0 = z@Wlin1.T+blin1)
  a        = dep[:,idx,idx-1] * hv_{idx-1}           (idx>=1)
  h_in     = 15*sigmoid(b_gate)*b_map + sigmoid(a@Wg.T+bg) * (a@Wm.T+bm)   (h_in=gs0 at idx=0)
  gru gates from h_in and x_idx -> hv_idx
  d1[idx] = hv.w1, d2[idx] = hv.w2
edges[i,j] = step(d1[i]+d2[j]+be) for j<=i-2; edges[i,i-1] = step(d1[i-1]+d2[i-1]+be).

All matmuls run in fp32 (fp32r is an 11-bit-mantissa format — too coarse for the
hard edge thresholds). Biases enter via a constant ones-row appended to the
stationary operand (row 117 of the K-tile-3 slice) and bias rows baked into the
weight layouts, so no separate bias matmuls are needed.

Sharding: pure data parallel, batch 512 -> 64 per core across 8 cores.
"""

import numpy as np

B, S, H, C = 512, 16, 501, 7
NCORES = 8
BC = B // NCORES  # 64 batch rows per core
KT = [128, 128, 128, 117]    # K tiles over H=501 (data rows)
KTL = [128, 128, 128, 125]   # lhsT/rhs rows (tile 3: 117 data + ones row + 7 x rows)
KOFF = [0, 128, 256, 384]
H3 = 3 * H  # 1503

# packed bias/constant row: b_gate | b_map | b_edge | b_ih[0:2H] | b_hh[0:2H] | b_ih_n
BO_GATE, BO_MAP, BO_BE = 0, H, 2 * H
BO_IHRU = 2 * H + 1
BO_HHRU = BO_IHRU + 2 * H
BO_IHN = BO_HHRU + 2 * H
BIAS_LEN = BO_IHN + H

_CACHE = {}


def _build_module():
    import concourse.bass as bass
    import concourse.bacc as bacc
    import concourse.mybir as mybir
    from concourse.tile import TileContext

    f32 = mybir.dt.float32
    Alu = mybir.AluOpType
    Act = mybir.ActivationFunctionType
    Axis = mybir.AxisListType

    nc = bacc.Bacc("TRN2", target_bir_lowering=False, debug=False,
                   enable_asserts=False, num_devices=NCORES)

    # ---- DRAM I/O ----
    d_zT4 = nc.dram_tensor("zT4", [128, 256], f32, kind="ExternalInput")
    d_wlin = nc.dram_tensor("WlinT", [128, 4 * H], f32, kind="ExternalInput")
    d_wg = nc.dram_tensor("WgT", [128, 4 * H], f32, kind="ExternalInput")
    d_wm = nc.dram_tensor("WmT", [128, 4 * H], f32, kind="ExternalInput")
    d_whh = nc.dram_tensor("WhhT", [128, 4 * H3], f32, kind="ExternalInput")
    d_wih = nc.dram_tensor("WihT", [8, H3], f32, kind="ExternalInput")
    d_wv = nc.dram_tensor("WvT", [128, 4 * C], f32, kind="ExternalInput")
    d_w12 = nc.dram_tensor("w12", [128, 8], f32, kind="ExternalInput")
    d_xT = nc.dram_tensor("xT", [8, S * BC], f32, kind="ExternalInput")
    d_s4 = nc.dram_tensor("S4r", [128, 15 * 256], f32, kind="ExternalInput")
    d_eye = nc.dram_tensor("eye64", [64, 64], f32, kind="ExternalInput")
    d_mask = nc.dram_tensor("maskOD", [64, 256], f32, kind="ExternalInput")
    d_bias = nc.dram_tensor("BIASROW", [1, BIAS_LEN], f32, kind="ExternalInput")
    d_ones = nc.dram_tensor("onesrow", [1, 64], f32, kind="ExternalInput")
    d_zero8 = nc.dram_tensor("zero8", [8, 64], f32, kind="ExternalInput")
    d_odep = nc.dram_tensor("out_dep", [BC, 256], f32, kind="ExternalOutput")
    d_oenc = nc.dram_tensor("out_enc", [BC, S * C], f32, kind="ExternalOutput")

    def bcast(dram_handle, col0, ncols, nparts):
        ap = dram_handle.ap()
        return bass.AP(tensor=ap.tensor, offset=ap.offset + col0,
                       ap=[[0, nparts], [1, ncols]])

    with TileContext(nc) as tc:
        with (
            tc.tile_pool(name="const", bufs=1) as cp,
            tc.tile_pool(name="work", bufs=2) as wp,
            tc.tile_pool(name="psum", bufs=1, space="PSUM") as pp,
        ):
            # ---- constants into SBUF ----
            t_wlin = cp.tile([128, 4 * H], f32, name="t_wlin")
            t_wg = cp.tile([128, 4 * H], f32, name="t_wg")
            t_wm = cp.tile([128, 4 * H], f32, name="t_wm")
            t_whh = cp.tile([128, 4 * H3], f32, name="t_whh")
            t_wih = cp.tile([8, H3], f32, name="t_wih")
            t_wv = cp.tile([128, 4 * C], f32, name="t_wv")
            t_w12 = cp.tile([128, 8], f32, name="t_w12")
            t_xT = cp.tile([8, S * BC], f32, name="t_xT")
            t_s4 = cp.tile([128, 15 * 256], f32, name="t_s4")
            t_zT4 = cp.tile([128, 256], f32, name="t_zT4")
            t_eye = cp.tile([64, 64], f32, name="t_eye")
            t_mask = cp.tile([64, 256], f32, name="t_mask")
            t_bias = cp.tile([1, BIAS_LEN], f32, name="t_bias")
            t_c15 = cp.tile([64, H], f32, name="t_c15")
            t_nbe = cp.tile([64, 1], f32, name="t_nbe")
            t_enc = cp.tile([BC, S * C], f32, name="t_enc")
            t_d1 = cp.tile([64, 16], f32, name="t_d1")
            t_d2 = cp.tile([64, 16], f32, name="t_d2")
            t_ed = cp.tile([64, 256], f32, name="t_ed")
            t_bg = cp.tile([64, H], f32, name="t_bg")
            t_bm = cp.tile([64, H], f32, name="t_bm")
            t_bet = cp.tile([64, 1], f32, name="t_bet")
            t_bsc = cp.tile([1, 2 * H], f32, name="t_bsc")
            # persistent transposed-activation buffers (row 117 of slice 3 = ones)
            t_gsA = cp.tile([128, 256], f32, name="t_gsA")
            t_gsB = cp.tile([128, 256], f32, name="t_gsB")
            t_gsC = cp.tile([128, 256], f32, name="t_gsC")
            t_hT = cp.tile([128, 256], f32, name="t_hT")
            t_aT = cp.tile([128, 256], f32, name="t_aT")

            nc.sync.dma_start(out=t_wlin[:, :], in_=d_wlin.ap())
            nc.sync.dma_start(out=t_wg[:, :], in_=d_wg.ap())
            nc.sync.dma_start(out=t_wm[:, :], in_=d_wm.ap())
            nc.sync.dma_start(out=t_whh[:, :], in_=d_whh.ap())
            nc.sync.dma_start(out=t_wih[:, :], in_=d_wih.ap())
            nc.sync.dma_start(out=t_wv[:, :], in_=d_wv.ap())
            nc.sync.dma_start(out=t_w12[:, :], in_=d_w12.ap())
            nc.sync.dma_start(out=t_xT[:, :], in_=d_xT.ap())
            nc.sync.dma_start(out=t_s4[:, :], in_=d_s4.ap())
            nc.sync.dma_start(out=t_zT4[:, :], in_=d_zT4.ap())
            nc.sync.dma_start(out=t_eye[:, :], in_=d_eye.ap())
            nc.sync.dma_start(out=t_mask[:, :], in_=d_mask.ap())
            nc.sync.dma_start(out=t_bias[:, :], in_=d_bias.ap())
            nc.sync.dma_start(out=t_bg[:, :], in_=bcast(d_bias, BO_GATE, H, 64))
            nc.sync.dma_start(out=t_bm[:, :], in_=bcast(d_bias, BO_MAP, H, 64))
            nc.sync.dma_start(out=t_bet[:, :], in_=bcast(d_bias, BO_BE, 1, 64))
            for t in (t_gsA, t_gsB, t_gsC, t_hT, t_aT):
                nc.sync.dma_start(out=t[117:118, 192:256], in_=d_ones.ap())
            for t in (t_gsB, t_gsC, t_aT):
                nc.sync.dma_start(out=t[118:126, 192:256], in_=d_zero8.ap())
            # gsA doubles as hT at idx 0: rows 118:125 carry x_0
            nc.sync.dma_start(out=t_gsA[118:125, 192:256], in_=d_xT.ap()[0:7, 0:64])
            nc.sync.dma_start(out=t_gsA[125:126, 192:256], in_=d_zero8.ap()[0:1, :])

            nc.vector.memset(t_d1[:, :], 0.0)
            nc.vector.memset(t_d2[:, :], 0.0)

            # fold b_ih(r,u)+b_hh(r,u) into the hh-weight bias row (row 117 of K-tile 3)
            nc.vector.tensor_tensor(out=t_bsc[0:1, :], in0=t_bias[0:1, BO_IHRU:BO_IHRU + 2 * H],
                                    in1=t_bias[0:1, BO_HHRU:BO_HHRU + 2 * H], op=Alu.add)
            nc.sync.dma_start(out=t_whh[117:118, 3 * H3:3 * H3 + 2 * H], in_=t_bsc[0:1, :])


            # C15 = 15*sigmoid(b_gate)*b_map  (broadcast over 64 partitions)
            sg0 = wp.tile([64, H], f32, tag="sg")
            nc.scalar.activation(out=sg0[:, :], in_=t_bg[:, :], func=Act.Sigmoid)
            nc.vector.tensor_tensor(out=t_c15[:, :], in0=sg0[:, :], in1=t_bm[:, :], op=Alu.mult)
            nc.vector.tensor_scalar(out=t_c15[:, :], in0=t_c15[:, :],
                                    scalar1=float(S - 1), scalar2=None, op0=Alu.mult)
            # nbe = -b_edge - 1e-7 (threshold incl. the f32 sigmoid-rounding window)
            nc.vector.tensor_scalar(out=t_nbe[:, :], in0=t_bet[:, :],
                                    scalar1=-1.0, scalar2=-1e-7, op0=Alu.mult, op1=Alu.add)

            def mm_group(psum_ap, pairs):
                for i, (l, r) in enumerate(pairs):
                    nc.tensor.matmul(psum_ap, l, r,
                                     start=(i == 0), stop=(i == len(pairs) - 1))

            def transpose_into(psum_t, src, dst):
                # dst: (128,256) persistent sbuf; writes rows 0:117 of slice3 only
                for k in range(4):
                    nc.tensor.transpose(psum_t[0:KT[k], 64 * k:64 * k + 64],
                                        src[:, KOFF[k]:KOFF[k] + KT[k]], t_eye[:, :])
                nc.vector.tensor_copy(out=dst[0:128, 0:192], in_=psum_t[0:128, 0:192])
                nc.vector.tensor_copy(out=dst[0:117, 192:256], in_=psum_t[0:117, 192:256])

            def lhs_sl(buf, k):
                return buf[0:KTL[k], 64 * k:64 * k + 64]

            def hh_pairs(hT, g):
                return [(lhs_sl(hT, k),
                         t_whh[0:KTL[k], H3 * k + H * g:H3 * k + H * g + H]) for k in range(4)]

            def w_pairs(buf, t_w, n):
                return [(lhs_sl(buf, k), t_w[0:KTL[k], n * k:n * k + n]) for k in range(4)]

            gs_bufs = [t_gsA, t_gsB, t_gsC]

            def emit_enc(eidx, enc_src):
                plog = pp.tile([64, 16], f32, tag="psml", bufs=2)
                mm_group(plog[:, 0:C], w_pairs(enc_src, t_wv, C))
                mx = wp.tile([64, 1], f32, tag="mx")
                nc.vector.tensor_reduce(out=mx[:, :], in_=plog[:, 0:C], axis=Axis.X, op=Alu.max)
                nc.vector.tensor_scalar(out=mx[:, :], in0=mx[:, :],
                                        scalar1=-0.5, scalar2=None, op0=Alu.mult)
                th = wp.tile([64, C], f32, tag="th")
                nc.scalar.activation(out=th[:, :], in_=plog[:, 0:C], func=Act.Tanh,
                                     bias=mx[:, :], scale=0.5)
                num = wp.tile([64, C], f32, tag="num")
                nc.vector.tensor_scalar(out=num[:, :], in0=th[:, :],
                                        scalar1=1.0, scalar2=None, op0=Alu.add)
                den = wp.tile([64, C], f32, tag="den")
                nc.vector.tensor_scalar(out=den[:, :], in0=th[:, :],
                                        scalar1=-1.0, scalar2=1.0, op0=Alu.mult, op1=Alu.add)
                rd = wp.tile([64, C], f32, tag="rd")
                nc.vector.reciprocal(out=rd[:, :], in_=den[:, :])
                ex = wp.tile([64, C], f32, tag="ex")
                nc.vector.tensor_tensor(out=ex[:, :], in0=num[:, :], in1=rd[:, :], op=Alu.mult)
                sm = wp.tile([64, 1], f32, tag="sm")
                nc.vector.tensor_reduce(out=sm[:, :], in_=ex[:, :], axis=Axis.X, op=Alu.add)
                rc = wp.tile([64, 1], f32, tag="rc")
                nc.vector.reciprocal(out=rc[:, :], in_=sm[:, :])
                nc.vector.tensor_scalar(out=t_enc[:, C * eidx:C * eidx + C], in0=ex[:, :],
                                        scalar1=rc[:, :], scalar2=None, op0=Alu.mult)

            pending_enc = None
            gsT_prev = None
            for idx in range(S):
                if idx == 0:
                    pgm = pp.tile([128, 512], f32, tag="pgm")
                    mm_group(pgm[0:64, 0:H], [(t_zT4[0:KTL[k], 64 * k:64 * k + 64],
                                               t_wlin[0:KTL[k], H * k:H * k + H]) for k in range(4)])
                    h_in = wp.tile([64, H], f32, tag="h_in")
                    nc.vector.tensor_copy(out=h_in[:, :], in_=pgm[0:64, 0:H])
                    pt = pp.tile([128, 256], f32, tag="pT", bufs=2)
                    gsT = t_gsA
                    transpose_into(pt, h_in, gsT)
                    hT = gsT  # h_in == gs at idx 0
                else:
                    gsT = None
                    # a^T = gsT_prev * s  (s = dep[:,idx,idx-1] broadcast, host-prepared)
                    s0 = 256 * (idx - 1)
                    nc.vector.tensor_tensor(out=t_aT[0:128, 0:192], in0=gsT_prev[0:128, 0:192],
                                            in1=t_s4[0:128, s0:s0 + 192], op=Alu.mult)
                    nc.vector.tensor_tensor(out=t_aT[0:117, 192:256], in0=gsT_prev[0:117, 192:256],
                                            in1=t_s4[0:117, s0 + 192:s0 + 256], op=Alu.mult)
                    pgm = pp.tile([128, 512], f32, tag="pgm")
                    gp = w_pairs(t_aT, t_wg, H)
                    mp = w_pairs(t_aT, t_wm, H)
                    for k in range(4):  # interleave: col-groups 0:64 / 64:128 run concurrently
                        nc.tensor.matmul(pgm[0:64, 0:H], gp[k][0], gp[k][1],
                                         start=(k == 0), stop=(k == 3),
                                         skip_group_check=True)
                        nc.tensor.matmul(pgm[64:128, 0:H], mp[k][0], mp[k][1],
                                         start=(k == 0), stop=(k == 3),
                                         skip_group_check=True)
                    sg = wp.tile([64, H], f32, tag="sg")
                    nc.scalar.activation(out=sg[:, :], in_=pgm[0:64, 0:H], func=Act.Sigmoid)
                    h_in = wp.tile([64, H], f32, tag="h_in")
                    nc.vector.tensor_tensor(out=h_in[:, :], in0=sg[:, :], in1=pgm[64:128, 0:H], op=Alu.mult)
                    nc.vector.tensor_tensor(out=h_in[:, :], in0=h_in[:, :], in1=t_c15[:, :], op=Alu.add)
                    pt = pp.tile([128, 256], f32, tag="pT", bufs=2)
                    hT = t_hT
                    transpose_into(pt, h_in, hT)

                # ---- GRU ----
                if idx > 0:
                    # stage x_idx into the spare K-tile-3 rows of hT (r/u gi comes free)
                    nc.sync.dma_start(out=t_hT[118:125, 192:256],
                                      in_=t_xT[0:7, BC * idx:BC * idx + 64])
                # pair [r || hn] first (both feed the tanh chain); u after (needed late)
                prh = pp.tile([128, 512], f32, tag="prh")
                rp = hh_pairs(hT, 0)
                hp = hh_pairs(hT, 2)
                for k in range(4):
                    nc.tensor.matmul(prh[0:64, 0:H], rp[k][0], rp[k][1],
                                     start=(k == 0), stop=(k == 3),
                                     skip_group_check=True)
                    nc.tensor.matmul(prh[64:128, 0:H], hp[k][0], hp[k][1],
                                     start=(k == 0), stop=(k == 3),
                                     skip_group_check=True)
                pui = pp.tile([64, 512], f32, tag="pui")
                mm_group(pui[0:64, 0:H], hh_pairs(hT, 1))
                pin = pp.tile([64, 512], f32, tag="pin")
                nc.tensor.matmul(pin[0:64, 0:H], t_xT[0:8, BC * idx:BC * idx + 64],
                                 t_wih[0:8, 2 * H:3 * H], start=True, stop=True)

                r = wp.tile([64, H], f32, tag="r")
                u = wp.tile([64, H], f32, tag="u")
                u1m = wp.tile([64, H], f32, tag="u1m")
                nc.scalar.activation(out=r[:, :], in_=prh[0:64, 0:H], func=Act.Sigmoid)
                t1 = wp.tile([64, H], f32, tag="t1")
                nc.vector.tensor_tensor(out=t1[:, :], in0=r[:, :], in1=prh[64:128, 0:H], op=Alu.mult)
                t2 = wp.tile([64, H], f32, tag="t2")
                nc.vector.tensor_tensor(out=t2[:, :], in0=t1[:, :],
                                        in1=pin[0:64, 0:H], op=Alu.add)
                n = wp.tile([64, H], f32, tag="n")
                nc.scalar.activation(out=n[:, :], in_=t2[:, :], func=Act.Tanh)
                nc.scalar.activation(out=u[:, :], in_=pui[0:64, 0:H], func=Act.Sigmoid)
                nc.scalar.activation(out=u1m[:, :], in_=pui[0:64, 0:H], func=Act.Sigmoid, scale=-1.0)
                w = wp.tile([64, H], f32, tag="w")
                nc.vector.tensor_tensor(out=w[:, :], in0=u[:, :], in1=h_in[:, :], op=Alu.mult)
                m1 = wp.tile([64, H], f32, tag="m1")
                nc.vector.tensor_tensor(out=m1[:, :], in0=u1m[:, :], in1=n[:, :], op=Alu.mult)
                hv = wp.tile([64, H], f32, tag="hv")
                nc.vector.tensor_tensor(out=hv[:, :], in0=m1[:, :], in1=w[:, :], op=Alu.add)

                # ---- transpose hv -> gsT (for next step / dots / next enc) ----
                pt2 = pp.tile([128, 256], f32, tag="pT", bufs=2)
                gsT_new = gs_bufs[(idx + 1) % 3]
                transpose_into(pt2, hv, gsT_new)

                # ---- dots d1,d2 ----
                psm = pp.tile([64, 16], f32, tag="psml", bufs=2)
                mm_group(psm[:, 8:10], w_pairs(gsT_new, t_w12, 2))
                nc.vector.tensor_copy(out=t_d1[:, idx:idx + 1], in_=psm[:, 8:9])
                nc.vector.tensor_copy(out=t_d2[:, idx:idx + 1], in_=psm[:, 9:10])

                # ---- enc softmax: emit the PREVIOUS step's (keeps this step's
                # chain ops ahead of it on the scheduler priority heap) ----
                if pending_enc is not None:
                    emit_enc(*pending_enc)
                pending_enc = (idx, gsT if idx == 0 else gsT_prev)

                # ---- edge-margin row idx: ED[:, 16*idx + j] = d1[idx] + d2[j] ----
                nc.vector.tensor_scalar(out=t_ed[:, 16 * idx:16 * idx + 16], in0=t_d2[:, :],
                                        scalar1=t_d1[:, idx:idx + 1], scalar2=None, op0=Alu.add)

                gsT_prev = gsT_new

            if pending_enc is not None:
                emit_enc(*pending_enc)

            # ---- finalize edges ----
            tth = cp.tile([64, 256], f32, name="tth")
            nc.vector.tensor_scalar(out=tth[:, :], in0=t_ed[:, :],
                                    scalar1=t_nbe[:, :], scalar2=None, op0=Alu.is_ge)
            nc.vector.tensor_tensor(out=tth[:, :], in0=tth[:, :], in1=t_mask[:, :], op=Alu.mult)
            sd = cp.tile([64, 16], f32, name="sd")
            nc.vector.tensor_tensor(out=sd[:, :], in0=t_d1[:, :], in1=t_d2[:, :], op=Alu.add)
            sdt = cp.tile([64, 16], f32, name="sdt")
            nc.vector.tensor_scalar(out=sdt[:, :], in0=sd[:, :],
                                    scalar1=t_nbe[:, :], scalar2=None, op0=Alu.is_ge)
            for i in range(1, 16):
                nc.vector.tensor_copy(out=tth[:, 17 * i - 1:17 * i], in_=sdt[:, i - 1:i])

            nc.sync.dma_start(out=d_odep.ap(), in_=tth[:, :])
            nc.sync.dma_start(out=d_oenc.ap(), in_=t_enc[:, :])

    nc.compile()
    return nc


def _host_prep(z, dep_graph, node_encoding, W_lin1, b_lin1, W_vert, b_vert,
               W_edge, b_edge, W_gate, b_gate, W_map, b_map, W_ih, b_ih, W_hh, b_hh):
    f = np.float32

    def ktiled(WT, bias=None):  # WT: (H, N) -> (128, 4N); bias lands at row 117 of tile 3
        N = WT.shape[1]
        out = np.zeros((128, 4 * N), f)
        for k in range(4):
            out[0:KT[k], N * k:N * (k + 1)] = WT[KOFF[k]:KOFF[k] + KT[k], :]
        if bias is not None:
            out[117, 3 * N:3 * N + N] = bias
        return out

    whhT = np.zeros((128, 4 * H3), f)
    whhsrc = W_hh.T.astype(f)  # (H, 3H)
    for k in range(4):
        whhT[0:KT[k], H3 * k:H3 * (k + 1)] = whhsrc[KOFF[k]:KOFF[k] + KT[k], :]
    whhT[117, 3 * H3:4 * H3] = b_hh  # r/u thirds get b_ih added on-device
    # spare rows 118:125 of K-tile 3 carry W_ih.T for the r/u thirds (x rows of lhsT)
    whhT[118:125, 3 * H3:3 * H3 + 2 * H] = W_ih.T[:, 0:2 * H]

    shared = {
        "WlinT": ktiled(W_lin1.T.astype(f), b_lin1),
        "WgT": ktiled(W_gate.T.astype(f), b_gate),
        "WmT": ktiled(W_map.T.astype(f), b_map),
        "WhhT": whhT,
        "WvT": ktiled(W_vert.T.astype(f), b_vert),
        "w12": ktiled(W_edge.reshape(2, H).T.astype(f)),
        "eye64": np.eye(64, dtype=f),
        "onesrow": np.ones((1, 64), f),
    }
    wih = np.zeros((8, H3), f)
    wih[0:7] = W_ih.T
    wih[7] = b_ih
    shared["WihT"] = wih
    mask = np.zeros((64, 256), f)
    for i in range(16):
        for j in range(16):
            if j <= i - 2:
                mask[:, 16 * i + j] = 1.0
    shared["maskOD"] = mask
    bias = np.zeros((1, BIAS_LEN), f)
    bias[0, BO_GATE:BO_GATE + H] = b_gate
    bias[0, BO_MAP:BO_MAP + H] = b_map
    bias[0, BO_BE] = np.asarray(b_edge).ravel()[0]
    bias[0, BO_IHRU:BO_IHRU + 2 * H] = b_ih[0:2 * H]
    bias[0, BO_HHRU:BO_HHRU + 2 * H] = b_hh[0:2 * H]
    bias[0, BO_IHN:BO_IHN + H] = b_ih[2 * H:3 * H]
    shared["BIASROW"] = bias
    shared["zero8"] = np.zeros((8, 64), f)

    in_maps = []
    for c in range(NCORES):
        sl = slice(c * BC, (c + 1) * BC)
        zc = np.asarray(z[sl], f)
        zt4 = np.zeros((128, 256), f)
        for k in range(4):
            zt4[0:KT[k], 64 * k:64 * k + 64] = zc[:, KOFF[k]:KOFF[k] + KT[k]].T
        zt4[117, 192:256] = 1.0  # ones row for b_lin1
        xt = np.zeros((8, S * BC), f)
        nec = np.asarray(node_encoding[sl], f)  # (64, S, C)
        for idx in range(S):
            xt[0:7, BC * idx:BC * (idx + 1)] = nec[:, idx, :].T
        xt[7] = 1.0
        depc = np.asarray(dep_graph[sl], f)
        s4 = np.zeros((128, 15 * 256), f)
        for i in range(1, S):
            sub = depc[:, i, i - 1]  # (64,)
            s4[:, 256 * (i - 1):256 * i] = np.tile(sub[None, :], (128, 4))
        m = dict(shared)
        m.update({"zT4": zt4, "xT": xt, "S4r": s4})
        in_maps.append(m)
    return in_maps


def kernel(**inputs):
    from concourse.bass_utils import run_bass_kernel_spmd

    if "nc" not in _CACHE:
        _CACHE["nc"] = _build_module()
    nc = _CACHE["nc"]
    in_maps = _host_prep(**inputs)
    res = run_bass_kernel_spmd(nc, in_maps, core_ids=list(range(NCORES)))
    dep_out = np.concatenate(
        [res.results[c]["out_dep"].reshape(BC, S, S) for c in range(NCORES)], axis=0)
    enc_out = np.concatenate(
        [res.results[c]["out_enc"].reshape(BC, S, C) for c in range(NCORES)], axis=0)
    return dep_out.astype(np.float32), enc_out.astype(np.float32)
